# revision 5
# baseline (speedup 1.0000x reference)
"""Tensor-parallel attention kernel for 8 trn2 NeuronCores.

Strategy (tensor-parallel over heads):
  - each core owns 2 of the 16 heads: wq/wk/wv sharded column-wise,
    QKV projection + rope + attention fused per batch, Q/K/V kept in
    SBUF (no DRAM spill),
  - per-core attention outputs (256 rows of attn^T) are AllGathered in
    half-sequence slices so the collectives pipeline with compute,
  - the output projection (wo sharded column-wise) is interleaved into
    later batch segments as gathered halves land, leaving only the last
    half-AllGather + two small chunks exposed at the end.

All matmul operands are bf16 (FWL weight loads + 1 cyc/row streaming);
accumulation stays fp32 in PSUM, softmax normalization math fp32.
Causal-diagonal score tiles are computed on the live q-suffix only.
"""

import os

import numpy as np

import concourse.bass as bass
import concourse.mybir as mybir
import concourse.tile as tile
from concourse import bacc
from concourse.bass_utils import run_bass_kernel_spmd

B, S, D, H = 4, 2048, 2048, 16
HD = D // H            # 128
N_CORES = 8
HPC = H // N_CORES     # heads per core = 2
T = B * S              # 8192 tokens
CW = HPC * HD          # per-core feature width = 256

TOK = 512              # q/token tile (free dim of matmuls)
KTILE = 128            # k-token tile (partition dim)
NTT = T // TOK         # 16 token tiles over all batches
NTB = S // TOK         # 4 token tiles per batch
NQT = S // TOK         # 4 q tiles per sequence
NKT = S // KTILE       # 16 k tiles per sequence
NKC = D // 128         # 16 contraction chunks for projections
HS = S // 2            # half sequence (AllGather slice)

F32 = mybir.dt.float32
BF16 = mybir.dt.bfloat16

_KERNEL_CACHE = {}
_SKIP_AG = bool(int(os.environ.get("KERNEL_SKIP_AG", "0")))  # diagnostic only


def _analyze_mask(mask):
    """Per (k_tile, q_tile): skip entirely-masked tiles, flag tiles needing
    the additive mask. Works for causal, all-zero, and arbitrary masks."""
    m = mask.reshape(S, S)  # [q, k]
    mt = m.reshape(NQT, TOK, NKT, KTILE)
    tmax = mt.max(axis=(1, 3))  # [qt, kt]
    tmin = mt.min(axis=(1, 3))
    skip = tmax <= -1e8                      # exp underflows to exactly 0
    need = (~skip) & ((tmin != 0) | (tmax != 0))
    return skip.T, need.T                    # [kt, qt]


def _build(skip, need, n_mask_tiles, mask_uid, suffix_j):
    """Build the per-core Bass program. skip/need: [NKT, NQT] bool;
    mask_uid maps (kt, qt) -> index into the deduplicated mask-tile stack;
    suffix_j maps canonical causal-diagonal (kt, qt) -> j (q-suffix offset
    j*KTILE inside the q tile; first KTILE suffix cols triangular)."""
    nc = bacc.Bacc("TRN2", target_bir_lowering=False, debug=False,
                   num_devices=N_CORES)

    # x pre-chunked on host: xC[p, kc, t] = x[t, kc*128 + p]
    xC = nc.declare_dram_parameter("xC", [128, NKC, T], BF16, isOutput=False)
    # weights pre-chunked on host to [128, NKC*CW] (chunk kc at cols kc*CW)
    wq = nc.declare_dram_parameter("wq", [128, NKC * CW], BF16, isOutput=False)
    wk = nc.declare_dram_parameter("wk", [128, NKC * CW], BF16, isOutput=False)
    wv = nc.declare_dram_parameter("wv", [128, NKC * CW], BF16, isOutput=False)
    wo = nc.declare_dram_parameter("wo", [128, NKC * CW], BF16, isOutput=False)
    ropeC = nc.declare_dram_parameter("ropeC", [128, S], BF16, isOutput=False)
    ropeS = nc.declare_dram_parameter("ropeS", [128, S], BF16, isOutput=False)
    perm = nc.declare_dram_parameter("perm", [128, 128], BF16, isOutput=False)
    ident = nc.declare_dram_parameter("ident", [128, 128], BF16, isOutput=False)
    tri = nc.declare_dram_parameter("tri", [128, KTILE], BF16, isOutput=False)
    ones_col = nc.declare_dram_parameter("ones_col", [128, 1], BF16, isOutput=False)
    ones_row = nc.declare_dram_parameter("ones_row", [1, 128], BF16, isOutput=False)
    if n_mask_tiles:
        maskt = nc.declare_dram_parameter(
            "maskt", [n_mask_tiles, KTILE, TOK], BF16, isOutput=False)
    outT = nc.declare_dram_parameter("outT", [CW, T], F32, isOutput=True)

    # attention for q-tile qt can start once projections cover its last
    # active k-tile (for causal masks that is token-tile qt itself)
    maxtt = {}
    for qt in range(NQT):
        kts = [kt for kt in range(NKT) if not skip[kt, qt]]
        assert kts, f"fully masked q tile {qt}"
        maxtt[qt] = max(max(kts) // (TOK // KTILE), qt)
    # AllGather of half hf covers q tiles 2*hf, 2*hf+1
    ag_tl = [max(maxtt[2 * hf], maxtt[2 * hf + 1]) for hf in range(2)]

    inv_sqrt_hd = 1.0 / float(np.sqrt(HD))

    with tile.TileContext(nc) as tc:
        with tc.tile_pool(name="const", bufs=1) as const, \
             tc.tile_pool(name="dram", bufs=1, space="DRAM") as dram:
            # persistent SBUF constants
            wq_sb = const.tile([128, NKC * CW], BF16)
            wk_sb = const.tile([128, NKC * CW], BF16)
            wv_sb = const.tile([128, NKC * CW], BF16)
            wo_sb = const.tile([128, NKC * CW], BF16)
            for sb, dr in ((wq_sb, wq), (wk_sb, wk), (wv_sb, wv), (wo_sb, wo)):
                nc.sync.dma_start(sb[:], dr[:])
            C_sb = const.tile([128, S], BF16)
            S_sb = const.tile([128, S], BF16)
            nc.sync.dma_start(C_sb[:], ropeC[:])
            nc.sync.dma_start(S_sb[:], ropeS[:])
            perm_sb = const.tile([128, 128], BF16)
            ident_sb = const.tile([128, 128], BF16)
            tri_sb = const.tile([128, KTILE], BF16)
            onesc_sb = const.tile([128, 1], BF16)
            onesr_sb = const.tile([1, 128], BF16)
            nc.sync.dma_start(perm_sb[:], perm[:])
            nc.sync.dma_start(ident_sb[:], ident[:])
            nc.sync.dma_start(tri_sb[:], tri[:])
            nc.sync.dma_start(onesc_sb[:], ones_col[:])
            nc.sync.dma_start(onesr_sb[:], ones_row[:])
            preload_mask = 0 < n_mask_tiles <= 8
            mask_sb = None
            if preload_mask:
                mask_sb = const.tile([128, n_mask_tiles * TOK], BF16)
                for j in range(n_mask_tiles):
                    nc.sync.dma_start(
                        mask_sb[:, j * TOK:(j + 1) * TOK], maskt[j])

            # internal DRAM: AllGather buffers per (batch, half-sequence)
            ag_in = [[dram.tile([CW, HS], BF16, name=f"agin{b}_{hf}")
                      for hf in range(2)] for b in range(B)]
            ag_out = [[dram.tile([CW * N_CORES, HS], BF16,
                                 addr_space="Shared", name=f"agout{b}_{hf}")
                       for hf in range(2)] for b in range(B)]

            with tc.tile_pool(name="fx", bufs=2) as fx, \
                 tc.tile_pool(name="fbig", bufs=1) as fbig, \
                 tc.tile_pool(name="facc", bufs=1, space="PSUM") as facc, \
                 tc.tile_pool(name="fscr", bufs=3, space="PSUM") as fscr, \
                 tc.tile_pool(name="fo", bufs=2, space="PSUM") as fo, \
                 tc.tile_pool(name="fd", bufs=1, space="PSUM") as fd, \
                 tc.tile_pool(name="fsb", bufs=2) as fsb, \
                 tc.tile_pool(name="fex", bufs=4) as fex, \
                 tc.tile_pool(name="p3sb", bufs=4) as p3sb:

                def oproj_chunk(b, tl):
                    """Output projection for token tile tl of batch b,
                    reading the AllGathered half hf = tl // 2."""
                    hf, col0 = tl // 2, (tl % 2) * TOK
                    tt = NTB * b + tl
                    pss = [fo.tile([128, TOK], F32, tag="ops",
                                   name=f"po{m}") for m in range(HPC)]
                    for kc in range(NKC):
                        ach = p3sb.tile([128, TOK], BF16, tag="ach",
                                        name="ach")
                        eng = nc.scalar if kc % 2 else nc.sync
                        eng.dma_start(
                            ach[:],
                            ag_out[b][hf][128 * kc:128 * (kc + 1),
                                          col0:col0 + TOK])
                        st = (kc == 0)
                        sp = (kc == NKC - 1)
                        for m in range(HPC):
                            c0 = kc * CW + m * 128
                            nc.tensor.matmul(pss[m][:],
                                             wo_sb[:, c0:c0 + 128],
                                             ach[:], start=st, stop=sp)
                    for m in range(HPC):
                        osb = p3sb.tile([128, TOK], F32, tag="osb3",
                                        name="osb3")
                        nc.scalar.copy(osb[:], pss[m][:])
                        nc.sync.dma_start(
                            outT[128 * m:128 * (m + 1),
                                 TOK * tt:TOK * (tt + 1)], osb[:])

                # (b, tl) -> earliest global segment index it may be emitted
                ready_at = {}
                emitted = set()

                for b in range(B):
                    qT = [fbig.tile([128, S], BF16, tag=f"qT{h}", name=f"qT{h}")
                          for h in range(HPC)]
                    kT = [fbig.tile([128, S], BF16, tag=f"kT{h}", name=f"kT{h}")
                          for h in range(HPC)]
                    vnat = [fbig.tile([128, S], BF16, tag=f"vn{h}", name=f"vn{h}")
                            for h in range(HPC)]
                    for tl in range(NTB):
                        seg = NTB * b + tl
                        tt = seg
                        pos0 = tl * TOK
                        xall = fx.tile([128, NKC, TOK], BF16, tag="xall",
                                       name="xall")
                        nc.sync.dma_start(xall[:],
                                          xC[:, :, TOK * tt:TOK * (tt + 1)])
                        # Q pass then K pass; rope per head right after its
                        # psum accumulation completes (all rope data moves on
                        # DVE so the scalar engine stays free for exp)
                        for nm, wsb, dsts in (("q", wq_sb, qT),
                                              ("k", wk_sb, kT)):
                            pp = [facc.tile([128, TOK], F32, tag=f"pa{hh}",
                                            name=f"p{nm}{hh}")
                                  for hh in range(HPC)]
                            for hh in range(HPC):
                                for kc in range(NKC):
                                    c0 = kc * CW + hh * HD
                                    nc.tensor.matmul(pp[hh][:],
                                                     wsb[:, c0:c0 + HD],
                                                     xall[:, kc, :],
                                                     start=(kc == 0),
                                                     stop=(kc == NKC - 1))
                            for hh in range(HPC):
                                raw = fsb.tile([128, TOK], BF16, tag="raw",
                                               name="raw")
                                nc.vector.tensor_copy(raw[:], pp[hh][:])
                                swp = fscr.tile([128, TOK], F32, tag="scr",
                                                name="swp")
                                nc.tensor.matmul(swp[:], perm_sb[:], raw[:],
                                                 start=True, stop=True)
                                t1 = fsb.tile([128, TOK], BF16, tag="t1",
                                              name="t1")
                                nc.vector.tensor_mul(
                                    t1[:], raw[:], C_sb[:, pos0:pos0 + TOK])
                                t2 = fsb.tile([128, TOK], BF16, tag="t2",
                                              name="t2")
                                nc.vector.tensor_mul(
                                    t2[:], swp[:], S_sb[:, pos0:pos0 + TOK])
                                nc.vector.tensor_add(
                                    dsts[hh][:, pos0:pos0 + TOK],
                                    t1[:], t2[:])
                        # V pass for both heads (reuses pass-A psum slots)
                        pv = [facc.tile([128, TOK], F32, tag=f"pa{hh}",
                                        name=f"pav{hh}")
                              for hh in range(HPC)]
                        for hh in range(HPC):
                            for kc in range(NKC):
                                c0 = kc * CW + hh * HD
                                nc.tensor.matmul(pv[hh][:],
                                                 wv_sb[:, c0:c0 + HD],
                                                 xall[:, kc, :],
                                                 start=(kc == 0),
                                                 stop=(kc == NKC - 1))
                        for hh in range(HPC):
                            vts = fsb.tile([128, TOK], BF16, tag="vts",
                                           name="vts")
                            nc.vector.tensor_copy(vts[:], pv[hh][:])
                            # transpose 4 [128,128] blocks into vnat
                            for sub in range(TOK // KTILE):
                                kt = (pos0 // KTILE) + sub
                                tp = fscr.tile([128, KTILE], BF16, tag="scr",
                                               name="tp")
                                nc.tensor.matmul(
                                    tp[:],
                                    vts[:, KTILE * sub:KTILE * (sub + 1)],
                                    ident_sb[:], is_transpose=True,
                                    start=True, stop=True)
                                nc.vector.tensor_copy(
                                    vnat[hh][:, KTILE * kt:KTILE * (kt + 1)],
                                    tp[:])
                        # attention for every q-tile whose K/V coverage is
                        # now complete
                        for qt in range(NQT):
                          if maxtt[qt] != tl:
                            continue
                          for h in range(HPC):
                            kts = [kt for kt in range(NKT) if not skip[kt, qt]]
                            o_ps = fo.tile([128, TOK], F32, tag="ops",
                                           name="ops")
                            d_ps = fd.tile([1, TOK], F32, tag="dps",
                                           name="dps")
                            for j, kt in enumerate(kts):
                                st = (j == 0)
                                sp = (j == len(kts) - 1)
                                sj = suffix_j.get((kt, qt))
                                off = 0 if sj is None else sj * KTILE
                                W = TOK - off
                                assert not (st and off), \
                                    "first k-tile must cover the full q tile"
                                s_ps = fscr.tile([128, TOK], F32, tag="scr",
                                                 name="sps")
                                nc.tensor.matmul(
                                    s_ps[:, 0:W],
                                    kT[h][:, KTILE * kt:KTILE * (kt + 1)],
                                    qT[h][:, TOK * qt + off:TOK * (qt + 1)],
                                    start=True, stop=True)
                                ex = fex.tile([128, TOK], BF16, tag="ex",
                                              name="ex")
                                nc.scalar.activation(
                                    ex[:, 0:W], s_ps[:, 0:W],
                                    mybir.ActivationFunctionType.Exp,
                                    scale=inv_sqrt_hd)
                                if sj is not None:
                                    # triangular boundary: mask first KTILE
                                    # suffix cols in place on DVE
                                    nc.vector.tensor_mul(
                                        ex[:, 0:KTILE], ex[:, 0:KTILE],
                                        tri_sb[:])
                                elif need[kt, qt]:
                                    # general multiplicative mask tile
                                    mj = mask_uid[(kt, qt)]
                                    if preload_mask:
                                        msrc = mask_sb[:, mj * TOK:
                                                       (mj + 1) * TOK]
                                    else:
                                        mld = fsb.tile([128, TOK], BF16,
                                                       tag="mld", name="mld")
                                        nc.sync.dma_start(mld[:], maskt[mj])
                                        msrc = mld[:]
                                    nc.vector.tensor_mul(ex[:, 0:TOK],
                                                         ex[:, 0:TOK], msrc)
                                nc.tensor.matmul(
                                    o_ps[:, off:TOK],
                                    vnat[h][:, KTILE * kt:KTILE * (kt + 1)],
                                    ex[:, 0:W], start=st, stop=sp)
                                nc.tensor.matmul(d_ps[:, off:TOK],
                                                 onesc_sb[:], ex[:, 0:W],
                                                 start=st, stop=sp)
                            # normalization: broadcast denom via PE, then
                            # fast approx reciprocal on all 128 lanes
                            d_bf = fsb.tile([1, TOK], BF16, tag="dbf",
                                            name="dbf")
                            nc.vector.tensor_copy(d_bf[:], d_ps[:])
                            bc_ps = fscr.tile([128, TOK], F32, tag="scr",
                                              name="bc")
                            nc.tensor.matmul(bc_ps[:], onesr_sb[:], d_bf[:],
                                             start=True, stop=True)
                            rec = fsb.tile([128, TOK], F32, tag="rec",
                                           name="rec")
                            nc.vector.reciprocal_approx_fast(rec[:], bc_ps[:])
                            o_sb = fsb.tile([128, TOK], BF16, tag="osb",
                                            name="osb")
                            nc.vector.tensor_mul(o_sb[:], o_ps[:], rec[:])
                            nc.sync.dma_start(
                                ag_in[b][qt // 2][128 * h:128 * (h + 1),
                                                  (qt % 2) * TOK:
                                                  (qt % 2 + 1) * TOK],
                                o_sb[:])
                        # launch half-sequence AllGathers as soon as their
                        # q tiles are done
                        for hf in range(2):
                            if ag_tl[hf] == tl and not _SKIP_AG:
                                nc.gpsimd.collective_compute(
                                    "AllGather", mybir.AluOpType.bypass,
                                    ins=[ag_in[b][hf].opt()],
                                    outs=[ag_out[b][hf].opt()],
                                    replica_groups=[list(range(N_CORES))],
                                )
                                for tl2 in (2 * hf, 2 * hf + 1):
                                    ready_at[(b, tl2)] = seg + 1
                        # emit output-projection chunks whose AllGather has
                        # had a segment of lead time
                        for key in sorted(ready_at):
                            if ready_at[key] <= seg and key not in emitted:
                                emitted.add(key)
                                oproj_chunk(*key)
                # drain the remaining output-projection chunks
                for bb in range(B):
                    for tl2 in range(NTB):
                        if (bb, tl2) not in emitted:
                            oproj_chunk(bb, tl2)

    nc.compile()
    return nc


def prepare(x, wq, wk, wv, wo, freqs_cos, freqs_sin, mask, cache_k, cache_v,
            start_pos):
    """Compile (cached) and build per-core input maps."""
    assert int(start_pos) == 0, "kernel compiled for start_pos == 0"
    x = np.asarray(x, dtype=np.float32)
    wq = np.asarray(wq, dtype=np.float32)
    wk = np.asarray(wk, dtype=np.float32)
    wv = np.asarray(wv, dtype=np.float32)
    wo = np.asarray(wo, dtype=np.float32)
    fc = np.asarray(freqs_cos, dtype=np.float32)
    fs = np.asarray(freqs_sin, dtype=np.float32)
    mask = np.asarray(mask, dtype=np.float32)

    import ml_dtypes
    bf16 = ml_dtypes.bfloat16
    skip, need = _analyze_mask(mask)
    with np.errstate(under="ignore", over="ignore"):
        mT16 = np.exp(mask.reshape(S, S).T.astype(np.float64)).astype(
            np.float32)  # [k, q] multiplicative mask
    # canonical causal-diagonal tiles: cols < j*KTILE dead, then triangular
    suffix_j = {}
    kl = np.arange(KTILE)[:, None]
    ql = np.arange(TOK)[None, :]
    for qt in range(NQT):
        for jj in range(TOK // KTILE):
            kt = (TOK // KTILE) * qt + jj
            if kt >= NKT or not need[kt, qt]:
                continue
            pat = (ql >= jj * KTILE + kl).astype(np.float32)
            blk = mT16[KTILE * kt:KTILE * (kt + 1), TOK * qt:TOK * (qt + 1)]
            if np.array_equal(blk, pat):
                suffix_j[(kt, qt)] = jj
    uniq, mask_uid, tiles = {}, {}, []
    for kt in range(NKT):
        for qt in range(NQT):
            if need[kt, qt] and (kt, qt) not in suffix_j:
                tl16 = np.ascontiguousarray(
                    mT16[KTILE * kt:KTILE * (kt + 1),
                         TOK * qt:TOK * (qt + 1)])
                hkey = tl16.tobytes()
                if hkey not in uniq:
                    uniq[hkey] = len(tiles)
                    tiles.append(tl16)
                mask_uid[(kt, qt)] = uniq[hkey]
    n_mask_tiles = len(tiles)

    key = (skip.tobytes(), need.tobytes(),
           tuple(sorted(mask_uid.items())), tuple(sorted(suffix_j.items())))
    if key not in _KERNEL_CACHE:
        _KERNEL_CACHE[key] = _build(skip, need, n_mask_tiles, mask_uid,
                                    suffix_j)
    nc = _KERNEL_CACHE[key]

    # host-side input marshalling
    # xC[p, kc, t] = x[t, kc*128 + p]
    xC = np.ascontiguousarray(
        x.reshape(T, NKC, 128).transpose(2, 1, 0)).astype(bf16)
    C = np.repeat(fc.T, 2, axis=0).astype(bf16)                  # [128, S]
    Ssg = np.repeat(fs.T, 2, axis=0).astype(np.float32)
    Ssg[0::2] *= -1.0
    Ssg = Ssg.astype(bf16)
    pm = np.zeros((128, 128), np.float32)
    idx = np.arange(0, 128, 2)
    pm[idx, idx + 1] = 1.0
    pm[idx + 1, idx] = 1.0
    pm = pm.astype(bf16)
    ident = np.eye(128, dtype=np.float32).astype(bf16)
    tri = (np.arange(TOK)[None, :KTILE] >= np.arange(KTILE)[:, None]
           ).astype(np.float32).astype(bf16)  # [k, q] upper-tri incl diag
    ones_col = np.ones((128, 1), np.float32).astype(bf16)
    ones_row = np.ones((1, 128), np.float32).astype(bf16)
    maskt = (np.ascontiguousarray(np.stack(tiles)).astype(bf16) if tiles
             else np.zeros((0, KTILE, TOK), bf16))

    def chunk_w(w):  # [D, CW] -> [128, NKC*CW]
        return np.ascontiguousarray(
            w.reshape(NKC, 128, CW).transpose(1, 0, 2).reshape(
                128, NKC * CW)).astype(bf16)

    in_maps = []
    for i in range(N_CORES):
        cols = slice(CW * i, CW * (i + 1))
        m = {
            "xC": xC,
            "wq": chunk_w(wq[:, cols]),
            "wk": chunk_w(wk[:, cols]),
            "wv": chunk_w(wv[:, cols]),
            "wo": chunk_w(wo[:, cols]),
            "ropeC": C, "ropeS": Ssg,
            "perm": pm, "ident": ident, "tri": tri,
            "ones_col": ones_col, "ones_row": ones_row,
        }
        if n_mask_tiles:
            m["maskt"] = maskt
        in_maps.append(m)
    return nc, in_maps


def assemble(results):
    outT = np.concatenate([results[i]["outT"] for i in range(N_CORES)],
                          axis=0)  # [D, T]
    return np.ascontiguousarray(outT.T).reshape(B, S, D)


def kernel(**inputs):
    nc, in_maps = prepare(**inputs)
    res = run_bass_kernel_spmd(nc, in_maps, list(range(N_CORES)))
    return assemble(res.results)
